# revision 36
# baseline (speedup 1.0000x reference)
# Trainium2 Bass kernel for nn_DecoderAttention (gnn_message_passing), v4.
# Self-contained: host-side prep (numpy) + bass/Tile device kernel + SPMD run.
#
# Sharding: 8 cores = batch(2) x H-quarters(4). Each core: one batch element,
# 24 output rows (+1 halo row each side), all 4 contexts.
#
# v4 design: softmax probabilities are transposed to pT [18, PW], then the
# per-delta broadcast to 128 channel-partitions goes through a replicating
# SBUF->SBUF DMA (stride-0 source AP) producing bf16 psb tiles in SBUF; the
# products multiply runs as an all-SBUF-bf16 tensor_tensor in the DVE 2x
# mode (or on Pool for selected deltas). Pair 1 is processed in two pixel
# halves so its products pipeline inside the pair-1 tanh window instead of
# serializing after it. Input loads are issued from three different engine
# queues in small chunks for a fast lead-in. w_attn stays folded into the
# values conv / fold matmuls; everything accumulates in one PSUM tile per
# output pixel-tile; output is written pixel-major bf16.
import numpy as np
import ml_dtypes

E, D, K, B, H, W = 64, 128, 4, 2, 96, 96
RP = 98
ROWS_OUT = 24
ROWS_IN = ROWS_OUT + 2
NBUF = 2656               # in-buffer width (26*98=2548, padded for +-99 shifts)
OB = RP + 1               # out-window base = 99
NW = ROWS_OUT * RP - 2    # 2350 real out-window pixels
T = 19                    # pixel tiles of 128 (19*128 = 2432 >= NW)
PW = T * 128              # 2432
TSPLIT = 10               # pair-1 half split: H1 = tiles [0,10), H2 = [10,19)
NCORES = 8

BF = ml_dtypes.bfloat16
DELTAS = [(r, c) for r in range(3) for c in range(3)]
DOFF = [(r - 1) * RP + (c - 1) for r, c in DELTAS]

# chunk grids
QCH = [(i * 512, min(512, PW - i * 512)) for i in range(5)]       # q over PW
KCH = [(i * 512, min(512, NBUF - i * 512)) for i in range(6)]     # keys

# evac engine assignment (psum f32 -> sbuf bf16): only 'dve' | 'act'
EVAC_Q = ["dve", "dve", "act", "dve", "act"]
EVAC_K0 = ["act", "dve", "act", "dve", "act", "dve"]
EVAC_K1 = ["dve", "dve", "dve", "dve", "dve", "dve"]
# products multiply engine: [pair0 9] [pair1-H1 9] [pair1-H2 9]
PM0 = ["dve", "dve", "dve", "dve", "dve", "dve", "dve", "pool", "pool"]
PM1A = ["dve", "dve", "dve", "dve", "dve", "dve", "dve", "dve", "pool"]
PM1B = ["dve", "dve", "dve", "dve", "dve", "dve", "dve", "dve", "dve"]

_CACHE = {}


def _bf16(x):
    return np.ascontiguousarray(np.asarray(x, np.float32).astype(BF))


def _f32(x):
    return np.ascontiguousarray(np.asarray(x, np.float32))


# ---------------------------------------------------------------- host prep
def _pad_to_buf(img, r0):
    """img [C, H, W] -> [C, NBUF] zero-padded halo'd row-pitch-98 buffer."""
    C = img.shape[0]
    out = np.zeros((C, NBUF), np.float32)
    lr = np.arange(ROWS_IN)
    gr = r0 - 1 + lr
    ok = (gr >= 0) & (gr < H)
    for i in np.nonzero(ok)[0]:
        out[:, i * RP + 1: i * RP + 1 + W] = img[:, gr[i], :]
    return out


def _prep_weights(inputs):
    w_enc, b_enc = _f32(inputs["w_enc"]), _f32(inputs["b_enc"])
    w_dec, b_dec = _f32(inputs["w_dec"]), _f32(inputs["b_dec"])
    w_agg = _f32(inputs["w_agg"])
    w_val, b_val = _f32(inputs["w_val"]), _f32(inputs["b_val"])
    w_attn, b_attn = _f32(inputs["w_attn"]), _f32(inputs["b_attn"])

    Wdec_dup = np.empty((128, 128), np.float32)
    for m in range(128):
        Wdec_dup[:, m] = w_dec[m % 64, :]
    WencB = np.zeros((128, 128), np.float32)
    WencB[:64, :64] = w_enc.T
    WencB[64:, 64:] = w_enc.T
    Wagg2 = np.zeros((128, 2), np.float32)
    Wagg2[:64, 0] = w_agg
    Wagg2[64:, 1] = w_agg
    I128 = np.eye(128, dtype=np.float32)
    # w_attn folded into the 3x3 values conv: [d, delta, o]
    WvalT = np.zeros((128, 9, 64), np.float32)
    for di, (r, c) in enumerate(DELTAS):
        WvalT[:, di, :] = (w_attn[:, :64] @ w_val[:, :, r, c]).T
    # fold matmuls rhs per pair: [(k*64+e), o] = w_attn[o, 64*(1+2p+k)+e]
    WfoldP = np.zeros((128, 2, 64), np.float32)
    for p in range(2):
        for k in range(2):
            blk = w_attn[:, 64 * (1 + 2 * p + k): 64 * (2 + 2 * p + k)]
            WfoldP[64 * k: 64 * (k + 1), p, :] = blk.T
    b_out = (w_attn[:, :64] @ b_val + b_attn).reshape(1, 64)
    b_qk = np.tile(b_dec + b_enc, 2).reshape(128, 1)

    # one packed [128, 1090] weight blob: Wenc|Wdec|Wagg | I128 | WvalT | WfoldP
    Wblob = np.zeros((128, 1090), np.float32)
    Wblob[:, 0:128] = WencB
    Wblob[:, 128:256] = Wdec_dup
    Wblob[:, 256:258] = Wagg2
    Wblob[:, 258:386] = I128
    Wblob[:, 386:962] = WvalT.reshape(128, 576)
    Wblob[:, 962:1090] = WfoldP.reshape(128, 128)
    return dict(
        Wblob=_bf16(Wblob), ones1=_bf16(np.ones((1, 128), np.float32)),
        brow=_bf16(b_out), b_qk=_f32(b_qk),
    )


def _prep_mask(r0):
    """mask_pix [128, T*18] bf16: per pixel tile t, partition P, col di*2+k."""
    s = np.arange(T * 128)
    pos = OB + s
    lr = pos // RP - 1
    c = pos % RP - 1
    valid_px = (s < NW) & (c >= 0) & (c < W) & (lr >= 0) & (lr < ROWS_OUT)
    m = np.zeros((T * 128, 9), np.float32)
    for di, (r, cc) in enumerate(DELTAS):
        dr, dc = r - 1, cc - 1
        ok = valid_px & (r0 + lr + dr >= 0) & (r0 + lr + dr < H) \
            & (c + dc >= 0) & (c + dc < W)
        m[:, di] = ok.astype(np.float32)
    # junk pixels: keep center neighbor on so Z > 0 (no inf/NaN downstream)
    m[~valid_px, 4] = 1.0
    m2 = np.repeat(m[:, :, None], 2, axis=2)          # [T*128, 9, 2]
    m2 = m2.reshape(T, 128, 18).transpose(1, 0, 2)    # [128, T, 18]
    return _bf16(m2.reshape(128, T * 18))


def host_prep(inputs):
    wt = _prep_weights(inputs)
    ctx = _f32(inputs["contexts"])
    dec = _f32(inputs["decoded_features"])
    cores = []
    for b in range(B):
        for q in range(4):
            r0 = q * ROWS_OUT
            ctx01 = np.zeros((128, NBUF), np.float32)
            ctx01[:64] = _pad_to_buf(ctx[0, b], r0)
            ctx01[64:] = _pad_to_buf(ctx[1, b], r0)
            ctx23 = np.zeros((128, NBUF), np.float32)
            ctx23[:64] = _pad_to_buf(ctx[2, b], r0)
            ctx23[64:] = _pad_to_buf(ctx[3, b], r0)
            core = dict(
                dec=_bf16(_pad_to_buf(dec[b], r0)),
                ctx01=_bf16(ctx01), ctx23=_bf16(ctx23),
                mask_pix=_prep_mask(r0),
            )
            core.update(wt)
            cores.append(core)
    return cores


# ---------------------------------------------------------------- bass build
def build_nc():
    import concourse.bacc as bacc
    import concourse.bass as bass
    import concourse.mybir as mybir
    import concourse.tile as tile

    f32d = mybir.dt.float32
    bf16d = mybir.dt.bfloat16
    AF = mybir.ActivationFunctionType
    OP = mybir.AluOpType

    nc = bacc.Bacc("TRN2", target_bir_lowering=False, debug=False,
                   num_devices=NCORES)

    d_dec = nc.dram_tensor("dec", [128, NBUF], bf16d, kind="ExternalInput").ap()
    d_ctx = [nc.dram_tensor("ctx01", [128, NBUF], bf16d, kind="ExternalInput").ap(),
             nc.dram_tensor("ctx23", [128, NBUF], bf16d, kind="ExternalInput").ap()]
    d_mask = nc.dram_tensor("mask_pix", [128, T * 18], bf16d,
                            kind="ExternalInput").ap()
    d_Wblob = nc.dram_tensor("Wblob", [128, 1090], bf16d,
                             kind="ExternalInput").ap()
    d_ones1 = nc.dram_tensor("ones1", [1, 128], bf16d, kind="ExternalInput").ap()
    d_brow = nc.dram_tensor("brow", [1, 64], bf16d, kind="ExternalInput").ap()
    d_bqk = nc.dram_tensor("b_qk", [128, 1], f32d, kind="ExternalInput").ap()
    d_out = nc.dram_tensor("out", [128, T * 64], bf16d, kind="ExternalOutput").ap()

    ENG = {}
    HW = TSPLIT * 128            # 1280
    HW2 = PW - HW                # 1152

    with tile.TileContext(nc) as tc:
        with tc.tile_pool(name="const", bufs=1) as const, \
             tc.tile_pool(name="big", bufs=1) as big, \
             tc.tile_pool(name="keysp", bufs=2) as keysp, \
             tc.tile_pool(name="work", bufs=6) as work, \
             tc.tile_pool(name="tanhp", bufs=12) as tanhp, \
             tc.tile_pool(name="ppF", bufs=3) as ppF, \
             tc.tile_pool(name="ppH", bufs=6) as ppH, \
             tc.tile_pool(name="psbF", bufs=4) as psbF, \
             tc.tile_pool(name="psbH", bufs=6) as psbH, \
             tc.tile_pool(name="psO", bufs=1, space="PSUM") as psO, \
             tc.tile_pool(name="psS", bufs=1, space="PSUM") as psS, \
             tc.tile_pool(name="psX", bufs=2, space="PSUM") as psX:

            ENG["dve"] = nc.vector
            ENG["act"] = nc.scalar
            ENG["pool"] = nc.gpsimd

            def loadn(pool, ap, shape, dtype, tag, n, eng):
                t = pool.tile(shape, dtype, tag=tag, name=tag)
                w = shape[1]
                step = (w + n - 1) // n
                for i in range(n):
                    c0 = i * step
                    cw = min(step, w - c0)
                    eng.dma_start(out=t[:, c0:c0 + cw], in_=ap[:, c0:c0 + cw])
                return t

            # --- loads spread across three issue queues for a fast lead-in
            dec_s = loadn(big, d_dec, [128, NBUF], bf16d, "dec", 3, nc.sync)
            ctx_s = [loadn(big, d_ctx[0], [128, NBUF], bf16d, "ctx01", 3,
                           nc.scalar), None]
            Wblob = loadn(const, d_Wblob, [128, 1090], bf16d, "Wblob", 1,
                          nc.gpsimd)
            bqk = loadn(const, d_bqk, [128, 1], f32d, "bqk", 1, nc.gpsimd)
            mask_s = loadn(const, d_mask, [128, T * 18], bf16d, "mask", 1,
                           nc.gpsimd)
            ctx_s[1] = loadn(big, d_ctx[1], [128, NBUF], bf16d, "ctx23", 2,
                             nc.gpsimd)
            ones1 = loadn(const, d_ones1, [1, 128], bf16d, "ones1", 1,
                          nc.gpsimd)
            brow = loadn(const, d_brow, [1, 64], bf16d, "brow", 1, nc.gpsimd)
            mask_v = mask_s.rearrange("q (t d) -> q t d", t=T)
            Wenc = Wblob[:, 0:128]
            Wdec = Wblob[:, 128:256]
            Wagg = Wblob[:, 256:258]
            I128 = Wblob[:, 258:386]
            WvalT = Wblob[:, 386:962].rearrange("q (a b) -> q a b", a=9)
            WfoldP = Wblob[:, 962:1090].rearrange("q (a b) -> q a b", a=2)

            # ---- output accumulator psum: [128px, T*64] f32
            psO_all = psO.tile([128, T * 64], f32d, tag="o", name="psO_all")

            def evac(dst, src, eng):
                if eng == "act":
                    nc.scalar.activation(out=dst, in_=src, func=AF.Identity)
                else:
                    ENG[eng].tensor_copy(dst, src)

            # ---- PE p-state warmup: dummy matmuls on a memset tile keep
            # the PE busy while loads land, so the real q/keys matmuls run
            # at full speed instead of the slow ramp p-state.
            warm = big.tile([128, 512], bf16d, tag="warm", name="warm")
            nc.vector.memset(warm, 0.0)
            for i in range(7):
                psw = psX.tile([128, 512], f32d, tag="x", name="psw")
                nc.tensor.matmul(psw[0:16, 0:512], warm[:, 0:16], warm,
                                 start=True, stop=True)

            # ---- queries -> q_dup [128, PW] bf16 (+ b_qk)
            q_dup = big.tile([128, PW], bf16d, tag="q_dup", name="q_dup")
            for i, (c0, cw) in enumerate(QCH):
                psq = psX.tile([128, 512], f32d, tag="x", name="psq")
                nc.tensor.matmul(psq[:, 0:cw], Wdec,
                                 dec_s[:, OB + c0: OB + c0 + cw],
                                 start=True, stop=True)
                if EVAC_Q[i] == "act":
                    nc.scalar.activation(out=q_dup[:, c0:c0 + cw],
                                         in_=psq[:, 0:cw], func=AF.Identity,
                                         bias=bqk, scale=1.0)
                else:
                    nc.vector.tensor_scalar(
                        out=q_dup[:, c0:c0 + cw], in0=psq[:, 0:cw],
                        scalar1=bqk[:, 0:1], scalar2=None, op0=OP.add)

            keys = [None, None]
            p_pixs = [None, None]
            pT_sbs = [None, None]

            def emit_keys_chunk(p, i, evac_eng):
                if keys[p] is None:
                    keys[p] = keysp.tile([128, NBUF], bf16d, tag="keys",
                                         name=f"keys{p}")
                c0, cw = KCH[i]
                psk = psX.tile([128, 512], f32d, tag="x", name="psk")
                nc.tensor.matmul(psk[:, 0:cw], Wenc,
                                 ctx_s[p][:, c0:c0 + cw],
                                 start=True, stop=True)
                evac(keys[p][:, c0:c0 + cw], psk[:, 0:cw], evac_eng)

            def emit_add(p, di, t0, t1):
                base = OB + DOFF[di]
                w = (t1 - t0) * 128
                tin = work.tile([128, PW], bf16d, tag="tin", name="tin")
                nc.vector.tensor_add(
                    tin[:, 0:w], q_dup[:, t0 * 128: t0 * 128 + w],
                    keys[p][:, base + t0 * 128: base + t0 * 128 + w])
                return tin

            def emit_tanh_scores(p, di, t0, t1, tin, ps, start, stop):
                w = (t1 - t0) * 128
                th = tanhp.tile([128, PW], bf16d, tag="tanh",
                                name=f"tanh{p}_{di}_{t0}")
                nc.scalar.activation(out=th[:, 0:w], in_=tin[:, 0:w],
                                     func=AF.Tanh)
                for t in range(t0, t1):
                    nc.tensor.matmul(
                        ps[:, t - t0, di, :],
                        th[:, (t - t0) * 128: (t - t0) * 128 + 128],
                        Wagg, start=(start and di == 0 and t == t0),
                        stop=(stop and di == 8 and t == t1 - 1))

            def emit_exp(p, t0, t1, ps):
                if p_pixs[p] is None:
                    p_pixs[p] = big.tile([128, T, 18], bf16d, tag=f"p_pix{p}",
                                         name=f"p_pix{p}")
                p_pix = p_pixs[p]
                nc.scalar.activation(
                    out=p_pix[:, t0:t1, :].rearrange("q a b -> q (a b)"),
                    in_=ps[:, 0:t1 - t0].rearrange("q a b c -> q (a b c)"),
                    func=AF.Exp)

            def emit_softmax_transpose(p, t0, t1):
                p_pix = p_pixs[p]
                nt = t1 - t0
                Zbuf = big.tile([128, T, 2], f32d, tag=f"Z{p}", name=f"Z{p}")
                rz = big.tile([128, T, 2], bf16d, tag=f"rz{p}", name=f"rz{p}")
                nc.vector.tensor_mul(p_pix[:, t0:t1], p_pix[:, t0:t1],
                                     mask_v[:, t0:t1])
                nc.vector.tensor_reduce(
                    out=Zbuf[:, t0:t1],
                    in_=p_pix[:, t0:t1].rearrange("q t (d k) -> q t k d", k=2),
                    axis=mybir.AxisListType.X, op=OP.add)
                with nc.allow_low_precision(reason="softmax recip in bf16"):
                    nc.vector.reciprocal(
                        out=rz[:, t0:t1].rearrange("q a b -> q (a b)"),
                        in_=Zbuf[:, t0:t1].rearrange("q a b -> q (a b)"))
                rzs = rz[:, t0:t1]
                rzb = bass.AP(tensor=rzs.tensor, offset=rzs.offset,
                              ap=[list(rzs.ap[0]), list(rzs.ap[1]), [0, 9],
                                  list(rzs.ap[-1])])
                nc.vector.tensor_mul(
                    p_pix[:, t0:t1].rearrange("q t (d k) -> q t d k", k=2),
                    p_pix[:, t0:t1].rearrange("q t (d k) -> q t d k", k=2),
                    rzb)
                if pT_sbs[p] is None:
                    pT_sbs[p] = big.tile([18, PW], bf16d, tag=f"pT{p}",
                                         name=f"pT{p}")
                pT_sb = pT_sbs[p]
                for b0 in range(t0, t1, 4):
                    n_in = min(4, t1 - b0)
                    pst = psX.tile([18, 512], bf16d, tag="x", name="psT")
                    for tt in range(n_in):
                        nc.tensor.matmul(pst[:, tt * 128:(tt + 1) * 128],
                                         p_pix[:, b0 + tt, :], I128,
                                         is_transpose=True,
                                         start=(tt == 0),
                                         stop=(tt == n_in - 1))
                    nc.vector.tensor_copy(
                        pT_sb[:, b0 * 128: b0 * 128 + n_in * 128],
                        pst[:, 0:n_in * 128])

            def emit_product(p, di, t0, t1, mult_eng, psb_pool, pr_pool, dma_eng):
                base = OB + DOFF[di]
                w = (t1 - t0) * 128
                pT_sb = pT_sbs[p]
                psb = psb_pool.tile([128, psb_pool._v4w], bf16d, tag="psb",
                                    name=f"psb{p}_{di}_{t0}")
                src = pT_sb[di * 2: di * 2 + 2, t0 * 128: t0 * 128 + w]
                rep = bass.AP(tensor=src.tensor, offset=src.offset,
                              ap=[list(src.ap[0]), [0, 64], list(src.ap[-1])])
                dma_eng.dma_start(out=psb[:, 0:w], in_=rep)
                prods = pr_pool.tile([128, pr_pool._v4w], bf16d, tag="prods",
                                     name=f"prods{p}_{di}_{t0}")
                ENG[mult_eng].tensor_mul(
                    prods[:, 0:w], psb[:, 0:w],
                    ctx_s[p][:, base + t0 * 128: base + t0 * 128 + w])
                for t in range(t0, t1):
                    nc.tensor.matmul(
                        psO_all[:, t * 64:(t + 1) * 64],
                        prods[:, (t - t0) * 128:(t - t0 + 1) * 128],
                        WfoldP[:, p, :], start=False, stop=False)

            ppF._v4w = PW
            ppH._v4w = HW
            psbF._v4w = PW
            psbH._v4w = HW

            # ---- scores psum tiles
            psS0 = psS.tile([128, T, 9, 2], f32d, tag="s0", name="psS0")
            psS1a = psS.tile([128, TSPLIT, 9, 2], f32d, tag="s1a", name="psS1a")
            psS1b = psS.tile([128, T - TSPLIT, 9, 2], f32d, tag="s1b",
                             name="psS1b")

            # ===== pipeline =====
            for i in range(6):
                emit_keys_chunk(0, i, EVAC_K0[i])
            for di in range(9):
                tin = emit_add(0, di, 0, T)
                emit_tanh_scores(0, di, 0, T, tin, psS0, True, True)
                if 1 <= di <= 6:
                    emit_keys_chunk(1, di - 1, EVAC_K1[di - 1])
            emit_exp(0, 0, T, psS0)
            tins_a = {}
            for di in range(5):
                tins_a[di] = emit_add(1, di, 0, TSPLIT)
            emit_softmax_transpose(0, 0, T)

            # values conv (w_attn folded) opens every psO region; PE-side,
            # emitted before the first folds
            for di in range(9):
                base = OB + DOFF[di]
                for t in range(T):
                    nc.tensor.matmul(
                        psO_all[:, t * 64:(t + 1) * 64],
                        dec_s[:, base + t * 128: base + (t + 1) * 128],
                        WvalT[:, di, :],
                        start=(di == 0 and t in (0, 8, 16)), stop=False)

            # pair0 products interleaved with pair1-H1 adds/tanh/scores
            for di in range(9):
                if di >= 5:
                    tins_a[di] = emit_add(1, di, 0, TSPLIT)
                emit_tanh_scores(1, di, 0, TSPLIT, tins_a[di], psS1a,
                                 True, True)
                emit_product(0, di, 0, T, PM0[di], psbF, ppF, nc.sync)
            emit_exp(1, 0, TSPLIT, psS1a)
            # two pair1-H2 adds emitted early so tanh1-H2 never starves
            tins_b = {}
            for di in range(2):
                tins_b[di] = emit_add(1, di, TSPLIT, T)
            emit_softmax_transpose(1, 0, TSPLIT)
            for di in range(9):
                if di >= 2:
                    tins_b[di] = emit_add(1, di, TSPLIT, T)
                emit_tanh_scores(1, di, TSPLIT, T, tins_b[di], psS1b,
                                 True, True)
                emit_product(1, di, 0, TSPLIT, PM1A[di], psbH, ppH, nc.sync)
            emit_exp(1, TSPLIT, T, psS1b)
            emit_softmax_transpose(1, TSPLIT, T)
            for di in range(9):
                emit_product(1, di, TSPLIT, T, PM1B[di], psbH, ppH, nc.sync)

            # ---- bias + leaky + out, pipelined per psum bank
            o_f = big.tile([128, T * 64], bf16d, tag="o_f", name="o_f")
            o_s2 = big.tile([128, T * 64], bf16d, tag="o_s2", name="o_s2")
            o_sb = big.tile([128, T * 64], bf16d, tag="o_sb", name="o_sb")
            for lo, hi in ((0, 8), (8, 16), (16, T)):
                for t in range(lo, hi):
                    nc.tensor.matmul(psO_all[:, t * 64:(t + 1) * 64],
                                     ones1, brow, start=False,
                                     stop=(t == hi - 1))
                r0, r1 = lo * 64, hi * 64
                nc.scalar.activation(out=o_f[:, r0:r1],
                                     in_=psO_all[:, r0:r1],
                                     func=AF.Identity)
                nc.vector.tensor_scalar(
                    out=o_s2[:, r0:r1], in0=o_f[:, r0:r1],
                    scalar1=0.2, scalar2=None, op0=OP.mult)
                nc.vector.tensor_max(o_sb[:, r0:r1], o_f[:, r0:r1],
                                     o_s2[:, r0:r1])
                nc.sync.dma_start(out=d_out[:, r0:r1], in_=o_sb[:, r0:r1])

    nc.compile()
    return nc


def _get_nc():
    if "nc" not in _CACHE:
        _CACHE["nc"] = build_nc()
    return _CACHE["nc"]


# ---------------------------------------------------------------- entry point
def _assemble(results):
    out = np.zeros((B, E, H, W), np.float32)
    s = np.arange(NW)
    pos = OB + s
    lr = pos // RP - 1
    c = pos % RP - 1
    sel = (c >= 0) & (c < W)
    for b in range(B):
        for q in range(4):
            res = np.asarray(results[b * 4 + q]["out"]).astype(np.float32)
            res = res.reshape(128, T, 64).transpose(1, 0, 2).reshape(T * 128, 64)
            out[b, :, q * ROWS_OUT + lr[sel], c[sel]] = res[s[sel]]
    return out


def kernel(**inputs):
    from concourse import bass_utils

    cores = host_prep(inputs)
    nc = _get_nc()
    res = bass_utils.run_bass_kernel_spmd(
        nc, [dict(c) for c in cores], core_ids=list(range(NCORES)))
    return _assemble(res.results)


# revision 38
# speedup vs baseline: 1.0265x; 1.0265x over previous
# Trainium2 Bass kernel for nn_DecoderAttention (gnn_message_passing), v4.
# Self-contained: host-side prep (numpy) + bass/Tile device kernel + SPMD run.
#
# Sharding: 8 cores = batch(2) x H-quarters(4). Each core: one batch element,
# 24 output rows (+1 halo row each side), all 4 contexts.
#
# v4 design: softmax probabilities are transposed to pT [18, PW], then the
# per-delta broadcast to 128 channel-partitions goes through a replicating
# SBUF->SBUF DMA (stride-0 source AP) producing bf16 psb tiles in SBUF; the
# products multiply runs as an all-SBUF-bf16 tensor_tensor in the DVE 2x
# mode (or on Pool for selected deltas). Pair 1 is processed in two pixel
# halves so its products pipeline inside the pair-1 tanh window instead of
# serializing after it. Input loads are issued from three different engine
# queues in small chunks for a fast lead-in. w_attn stays folded into the
# values conv / fold matmuls; everything accumulates in one PSUM tile per
# output pixel-tile; output is written pixel-major bf16.
import numpy as np
import ml_dtypes

E, D, K, B, H, W = 64, 128, 4, 2, 96, 96
RP = 98
ROWS_OUT = 24
ROWS_IN = ROWS_OUT + 2
NBUF = 2656               # in-buffer width (26*98=2548, padded for +-99 shifts)
OB = RP + 1               # out-window base = 99
NW = ROWS_OUT * RP - 2    # 2350 real out-window pixels
T = 19                    # pixel tiles of 128 (19*128 = 2432 >= NW)
PW = T * 128              # 2432
TSPLIT = 10               # pair-1 half split: H1 = tiles [0,10), H2 = [10,19)
NCORES = 8

BF = ml_dtypes.bfloat16
DELTAS = [(r, c) for r in range(3) for c in range(3)]
DOFF = [(r - 1) * RP + (c - 1) for r, c in DELTAS]

# chunk grids
QCH = [(i * 512, min(512, PW - i * 512)) for i in range(5)]       # q over PW
KCH = [(i * 512, min(512, NBUF - i * 512)) for i in range(6)]     # keys

# evac engine assignment (psum f32 -> sbuf bf16): only 'dve' | 'act'
EVAC_Q = ["dve", "dve", "act", "dve", "act"]
EVAC_K0 = ["act", "dve", "act", "dve", "act", "dve"]
EVAC_K1 = ["dve", "dve", "dve", "dve", "dve", "dve"]
# products multiply engine: [pair0 9] [pair1-H1 9] [pair1-H2 9]
PM0 = ["dve", "dve", "dve", "dve", "dve", "dve", "dve", "pool", "pool"]
PM1A = ["dve", "dve", "dve", "dve", "dve", "dve", "dve", "dve", "pool"]
PM1B = ["dve", "dve", "dve", "dve", "dve", "dve", "dve", "dve", "dve"]

_CACHE = {}


def _bf16(x):
    return np.ascontiguousarray(np.asarray(x, np.float32).astype(BF))


def _f32(x):
    return np.ascontiguousarray(np.asarray(x, np.float32))


# ---------------------------------------------------------------- host prep
def _pad_to_buf(img, r0):
    """img [C, H, W] -> [C, NBUF] zero-padded halo'd row-pitch-98 buffer."""
    C = img.shape[0]
    out = np.zeros((C, NBUF), np.float32)
    lr = np.arange(ROWS_IN)
    gr = r0 - 1 + lr
    ok = (gr >= 0) & (gr < H)
    for i in np.nonzero(ok)[0]:
        out[:, i * RP + 1: i * RP + 1 + W] = img[:, gr[i], :]
    return out


def _prep_weights(inputs):
    w_enc, b_enc = _f32(inputs["w_enc"]), _f32(inputs["b_enc"])
    w_dec, b_dec = _f32(inputs["w_dec"]), _f32(inputs["b_dec"])
    w_agg = _f32(inputs["w_agg"])
    w_val, b_val = _f32(inputs["w_val"]), _f32(inputs["b_val"])
    w_attn, b_attn = _f32(inputs["w_attn"]), _f32(inputs["b_attn"])

    Wdec_dup = np.empty((128, 128), np.float32)
    for m in range(128):
        Wdec_dup[:, m] = w_dec[m % 64, :]
    WencB = np.zeros((128, 128), np.float32)
    WencB[:64, :64] = w_enc.T
    WencB[64:, 64:] = w_enc.T
    Wagg2 = np.zeros((128, 2), np.float32)
    Wagg2[:64, 0] = w_agg
    Wagg2[64:, 1] = w_agg
    I128 = np.eye(128, dtype=np.float32)
    # w_attn folded into the 3x3 values conv: [d, delta, o]
    WvalT = np.zeros((128, 9, 64), np.float32)
    for di, (r, c) in enumerate(DELTAS):
        WvalT[:, di, :] = (w_attn[:, :64] @ w_val[:, :, r, c]).T
    # fold matmuls rhs per pair: [(k*64+e), o] = w_attn[o, 64*(1+2p+k)+e]
    WfoldP = np.zeros((128, 2, 64), np.float32)
    for p in range(2):
        for k in range(2):
            blk = w_attn[:, 64 * (1 + 2 * p + k): 64 * (2 + 2 * p + k)]
            WfoldP[64 * k: 64 * (k + 1), p, :] = blk.T
    b_out = (w_attn[:, :64] @ b_val + b_attn).reshape(1, 64)
    b_qk = np.tile(b_dec + b_enc, 2).reshape(128, 1)

    # one packed [128, 1090] weight blob: Wenc|Wdec|Wagg | I128 | WvalT | WfoldP
    Wblob = np.zeros((128, 1090), np.float32)
    Wblob[:, 0:128] = WencB
    Wblob[:, 128:256] = Wdec_dup
    Wblob[:, 256:258] = Wagg2
    Wblob[:, 258:386] = I128
    Wblob[:, 386:962] = WvalT.reshape(128, 576)
    Wblob[:, 962:1090] = WfoldP.reshape(128, 128)
    return dict(
        Wblob=_bf16(Wblob), ones1=_bf16(np.ones((1, 128), np.float32)),
        brow=_bf16(b_out), b_qk=_f32(b_qk),
    )


def _prep_mask(r0):
    """mask_pix [128, T*18] bf16: per pixel tile t, partition P, col di*2+k."""
    s = np.arange(T * 128)
    pos = OB + s
    lr = pos // RP - 1
    c = pos % RP - 1
    valid_px = (s < NW) & (c >= 0) & (c < W) & (lr >= 0) & (lr < ROWS_OUT)
    m = np.zeros((T * 128, 9), np.float32)
    for di, (r, cc) in enumerate(DELTAS):
        dr, dc = r - 1, cc - 1
        ok = valid_px & (r0 + lr + dr >= 0) & (r0 + lr + dr < H) \
            & (c + dc >= 0) & (c + dc < W)
        m[:, di] = ok.astype(np.float32)
    # junk pixels: keep center neighbor on so Z > 0 (no inf/NaN downstream)
    m[~valid_px, 4] = 1.0
    m2 = np.repeat(m[:, :, None], 2, axis=2)          # [T*128, 9, 2]
    m2 = m2.reshape(T, 128, 18).transpose(1, 0, 2)    # [128, T, 18]
    return _bf16(m2.reshape(128, T * 18))


def host_prep(inputs):
    wt = _prep_weights(inputs)
    ctx = _f32(inputs["contexts"])
    dec = _f32(inputs["decoded_features"])
    cores = []
    for b in range(B):
        for q in range(4):
            r0 = q * ROWS_OUT
            ctx01 = np.zeros((128, NBUF), np.float32)
            ctx01[:64] = _pad_to_buf(ctx[0, b], r0)
            ctx01[64:] = _pad_to_buf(ctx[1, b], r0)
            ctx23 = np.zeros((128, NBUF), np.float32)
            ctx23[:64] = _pad_to_buf(ctx[2, b], r0)
            ctx23[64:] = _pad_to_buf(ctx[3, b], r0)
            core = dict(
                dec=_bf16(_pad_to_buf(dec[b], r0)),
                ctx01=_bf16(ctx01), ctx23=_bf16(ctx23),
                mask_pix=_prep_mask(r0),
            )
            core.update(wt)
            cores.append(core)
    return cores


# ---------------------------------------------------------------- bass build
def build_nc():
    import concourse.bacc as bacc
    import concourse.bass as bass
    import concourse.mybir as mybir
    import concourse.tile as tile

    f32d = mybir.dt.float32
    bf16d = mybir.dt.bfloat16
    AF = mybir.ActivationFunctionType
    OP = mybir.AluOpType

    nc = bacc.Bacc("TRN2", target_bir_lowering=False, debug=False,
                   num_devices=NCORES)

    d_dec = nc.dram_tensor("dec", [128, NBUF], bf16d, kind="ExternalInput").ap()
    d_ctx = [nc.dram_tensor("ctx01", [128, NBUF], bf16d, kind="ExternalInput").ap(),
             nc.dram_tensor("ctx23", [128, NBUF], bf16d, kind="ExternalInput").ap()]
    d_mask = nc.dram_tensor("mask_pix", [128, T * 18], bf16d,
                            kind="ExternalInput").ap()
    d_Wblob = nc.dram_tensor("Wblob", [128, 1090], bf16d,
                             kind="ExternalInput").ap()
    d_ones1 = nc.dram_tensor("ones1", [1, 128], bf16d, kind="ExternalInput").ap()
    d_brow = nc.dram_tensor("brow", [1, 64], bf16d, kind="ExternalInput").ap()
    d_bqk = nc.dram_tensor("b_qk", [128, 1], f32d, kind="ExternalInput").ap()
    d_out = nc.dram_tensor("out", [128, T * 64], bf16d, kind="ExternalOutput").ap()

    ENG = {}
    HW = TSPLIT * 128            # 1280
    HW2 = PW - HW                # 1152

    with tile.TileContext(nc) as tc:
        with tc.tile_pool(name="const", bufs=1) as const, \
             tc.tile_pool(name="big", bufs=1) as big, \
             tc.tile_pool(name="keysp", bufs=2) as keysp, \
             tc.tile_pool(name="work", bufs=6) as work, \
             tc.tile_pool(name="tanhp", bufs=10) as tanhp, \
             tc.tile_pool(name="ppF", bufs=3) as ppF, \
             tc.tile_pool(name="ppH", bufs=6) as ppH, \
             tc.tile_pool(name="psbF", bufs=4) as psbF, \
             tc.tile_pool(name="psbH", bufs=8) as psbH, \
             tc.tile_pool(name="psO", bufs=1, space="PSUM") as psO, \
             tc.tile_pool(name="psS", bufs=1, space="PSUM") as psS, \
             tc.tile_pool(name="psX", bufs=2, space="PSUM") as psX:

            ENG["dve"] = nc.vector
            ENG["act"] = nc.scalar
            ENG["pool"] = nc.gpsimd

            def loadn(pool, ap, shape, dtype, tag, n, eng):
                t = pool.tile(shape, dtype, tag=tag, name=tag)
                w = shape[1]
                step = (w + n - 1) // n
                for i in range(n):
                    c0 = i * step
                    cw = min(step, w - c0)
                    eng.dma_start(out=t[:, c0:c0 + cw], in_=ap[:, c0:c0 + cw])
                return t

            # --- loads spread across three issue queues for a fast lead-in
            dec_s = loadn(big, d_dec, [128, NBUF], bf16d, "dec", 3, nc.sync)
            ctx_s = [loadn(big, d_ctx[0], [128, NBUF], bf16d, "ctx01", 3,
                           nc.scalar), None]
            Wblob = loadn(const, d_Wblob, [128, 1090], bf16d, "Wblob", 1,
                          nc.gpsimd)
            bqk = loadn(const, d_bqk, [128, 1], f32d, "bqk", 1, nc.gpsimd)
            mask_s = loadn(const, d_mask, [128, T * 18], bf16d, "mask", 1,
                           nc.gpsimd)
            ctx_s[1] = loadn(big, d_ctx[1], [128, NBUF], bf16d, "ctx23", 2,
                             nc.gpsimd)
            ones1 = loadn(const, d_ones1, [1, 128], bf16d, "ones1", 1,
                          nc.gpsimd)
            brow = loadn(const, d_brow, [1, 64], bf16d, "brow", 1, nc.gpsimd)
            mask_v = mask_s.rearrange("q (t d) -> q t d", t=T)
            Wenc = Wblob[:, 0:128]
            Wdec = Wblob[:, 128:256]
            Wagg = Wblob[:, 256:258]
            I128 = Wblob[:, 258:386]
            WvalT = Wblob[:, 386:962].rearrange("q (a b) -> q a b", a=9)
            WfoldP = Wblob[:, 962:1090].rearrange("q (a b) -> q a b", a=2)

            # ---- output accumulator psum: [128px, T*64] f32
            psO_all = psO.tile([128, T * 64], f32d, tag="o", name="psO_all")

            def evac(dst, src, eng):
                if eng == "act":
                    nc.scalar.activation(out=dst, in_=src, func=AF.Identity)
                else:
                    ENG[eng].tensor_copy(dst, src)

            # ---- PE p-state warmup: dummy matmuls on a memset tile keep
            # the PE busy while loads land, so the real q/keys matmuls run
            # at full speed instead of the slow ramp p-state.
            warm = big.tile([128, 512], bf16d, tag="warm", name="warm")
            nc.vector.memset(warm, 0.0)
            for i in range(7):
                psw = psX.tile([128, 512], f32d, tag="x", name="psw")
                nc.tensor.matmul(psw[0:16, 0:512], warm[:, 0:16], warm,
                                 start=True, stop=True)

            # ---- queries -> q_dup [128, PW] bf16 (+ b_qk)
            q_dup = big.tile([128, PW], bf16d, tag="q_dup", name="q_dup")
            for i, (c0, cw) in enumerate(QCH):
                psq = psX.tile([128, 512], f32d, tag="x", name="psq")
                nc.tensor.matmul(psq[:, 0:cw], Wdec,
                                 dec_s[:, OB + c0: OB + c0 + cw],
                                 start=True, stop=True)
                if EVAC_Q[i] == "act":
                    nc.scalar.activation(out=q_dup[:, c0:c0 + cw],
                                         in_=psq[:, 0:cw], func=AF.Identity,
                                         bias=bqk, scale=1.0)
                else:
                    nc.vector.tensor_scalar(
                        out=q_dup[:, c0:c0 + cw], in0=psq[:, 0:cw],
                        scalar1=bqk[:, 0:1], scalar2=None, op0=OP.add)

            keys = [None, None]
            p_pixs = [None, None]
            pT_sbs = [None, None]

            def emit_keys_chunk(p, i, evac_eng):
                if keys[p] is None:
                    keys[p] = keysp.tile([128, NBUF], bf16d, tag="keys",
                                         name=f"keys{p}")
                c0, cw = KCH[i]
                psk = psX.tile([128, 512], f32d, tag="x", name="psk")
                nc.tensor.matmul(psk[:, 0:cw], Wenc,
                                 ctx_s[p][:, c0:c0 + cw],
                                 start=True, stop=True)
                evac(keys[p][:, c0:c0 + cw], psk[:, 0:cw], evac_eng)

            def emit_add(p, di, t0, t1):
                base = OB + DOFF[di]
                w = (t1 - t0) * 128
                tin = work.tile([128, PW], bf16d, tag="tin", name="tin")
                nc.vector.tensor_add(
                    tin[:, 0:w], q_dup[:, t0 * 128: t0 * 128 + w],
                    keys[p][:, base + t0 * 128: base + t0 * 128 + w])
                return tin

            def emit_tanh_scores(p, di, t0, t1, tin, ps, start, stop):
                w = (t1 - t0) * 128
                th = tanhp.tile([128, PW], bf16d, tag="tanh",
                                name=f"tanh{p}_{di}_{t0}")
                nc.scalar.activation(out=th[:, 0:w], in_=tin[:, 0:w],
                                     func=AF.Tanh)
                for t in range(t0, t1):
                    nc.tensor.matmul(
                        ps[:, t - t0, di, :],
                        th[:, (t - t0) * 128: (t - t0) * 128 + 128],
                        Wagg, start=(start and di == 0 and t == t0),
                        stop=(stop and di == 8 and t == t1 - 1))

            def emit_exp(p, t0, t1, ps):
                if p_pixs[p] is None:
                    p_pixs[p] = big.tile([128, T, 18], bf16d, tag=f"p_pix{p}",
                                         name=f"p_pix{p}")
                p_pix = p_pixs[p]
                nc.scalar.activation(
                    out=p_pix[:, t0:t1, :].rearrange("q a b -> q (a b)"),
                    in_=ps[:, 0:t1 - t0].rearrange("q a b c -> q (a b c)"),
                    func=AF.Exp)

            def emit_softmax_transpose(p, t0, t1):
                p_pix = p_pixs[p]
                nt = t1 - t0
                Zbuf = big.tile([128, T, 2], f32d, tag=f"Z{p}", name=f"Z{p}")
                rz = big.tile([128, T, 2], bf16d, tag=f"rz{p}", name=f"rz{p}")
                nc.vector.tensor_mul(p_pix[:, t0:t1], p_pix[:, t0:t1],
                                     mask_v[:, t0:t1])
                nc.vector.tensor_reduce(
                    out=Zbuf[:, t0:t1],
                    in_=p_pix[:, t0:t1].rearrange("q t (d k) -> q t k d", k=2),
                    axis=mybir.AxisListType.X, op=OP.add)
                with nc.allow_low_precision(reason="softmax recip in bf16"):
                    nc.vector.reciprocal(
                        out=rz[:, t0:t1].rearrange("q a b -> q (a b)"),
                        in_=Zbuf[:, t0:t1].rearrange("q a b -> q (a b)"))
                rzs = rz[:, t0:t1]
                rzb = bass.AP(tensor=rzs.tensor, offset=rzs.offset,
                              ap=[list(rzs.ap[0]), list(rzs.ap[1]), [0, 9],
                                  list(rzs.ap[-1])])
                nc.vector.tensor_mul(
                    p_pix[:, t0:t1].rearrange("q t (d k) -> q t d k", k=2),
                    p_pix[:, t0:t1].rearrange("q t (d k) -> q t d k", k=2),
                    rzb)
                if pT_sbs[p] is None:
                    pT_sbs[p] = big.tile([18, PW], bf16d, tag=f"pT{p}",
                                         name=f"pT{p}")
                pT_sb = pT_sbs[p]
                for b0 in range(t0, t1, 4):
                    n_in = min(4, t1 - b0)
                    pst = psX.tile([18, 512], bf16d, tag="x", name="psT")
                    for tt in range(n_in):
                        nc.tensor.matmul(pst[:, tt * 128:(tt + 1) * 128],
                                         p_pix[:, b0 + tt, :], I128,
                                         is_transpose=True,
                                         start=(tt == 0),
                                         stop=(tt == n_in - 1))
                    nc.vector.tensor_copy(
                        pT_sb[:, b0 * 128: b0 * 128 + n_in * 128],
                        pst[:, 0:n_in * 128])

            def emit_product(p, di, t0, t1, mult_eng, psb_pool, pr_pool, dma_eng):
                base = OB + DOFF[di]
                w = (t1 - t0) * 128
                pT_sb = pT_sbs[p]
                psb = psb_pool.tile([128, psb_pool._v4w], bf16d, tag="psb",
                                    name=f"psb{p}_{di}_{t0}")
                src = pT_sb[di * 2: di * 2 + 2, t0 * 128: t0 * 128 + w]
                rep = bass.AP(tensor=src.tensor, offset=src.offset,
                              ap=[list(src.ap[0]), [0, 64], list(src.ap[-1])])
                dma_eng.dma_start(out=psb[:, 0:w], in_=rep)
                prods = pr_pool.tile([128, pr_pool._v4w], bf16d, tag="prods",
                                     name=f"prods{p}_{di}_{t0}")
                ENG[mult_eng].tensor_mul(
                    prods[:, 0:w], psb[:, 0:w],
                    ctx_s[p][:, base + t0 * 128: base + t0 * 128 + w])
                for t in range(t0, t1):
                    nc.tensor.matmul(
                        psO_all[:, t * 64:(t + 1) * 64],
                        prods[:, (t - t0) * 128:(t - t0 + 1) * 128],
                        WfoldP[:, p, :], start=False, stop=False)

            ppF._v4w = PW
            ppH._v4w = HW
            psbF._v4w = PW
            psbH._v4w = HW

            # ---- scores psum tiles
            psS0 = psS.tile([128, T, 9, 2], f32d, tag="s0", name="psS0")
            psS1a = psS.tile([128, TSPLIT, 9, 2], f32d, tag="s1a", name="psS1a")
            psS1b = psS.tile([128, T - TSPLIT, 9, 2], f32d, tag="s1b",
                             name="psS1b")

            # ===== pipeline =====
            for i in range(6):
                emit_keys_chunk(0, i, EVAC_K0[i])
            for di in range(9):
                tin = emit_add(0, di, 0, T)
                emit_tanh_scores(0, di, 0, T, tin, psS0, True, True)
                if 1 <= di <= 6:
                    emit_keys_chunk(1, di - 1, EVAC_K1[di - 1])
            emit_exp(0, 0, T, psS0)
            tins_a = {}
            for di in range(5):
                tins_a[di] = emit_add(1, di, 0, TSPLIT)
            emit_softmax_transpose(0, 0, T)

            # values conv (w_attn folded) opens every psO region; PE-side,
            # emitted before the first folds
            for di in range(9):
                base = OB + DOFF[di]
                for t in range(T):
                    nc.tensor.matmul(
                        psO_all[:, t * 64:(t + 1) * 64],
                        dec_s[:, base + t * 128: base + (t + 1) * 128],
                        WvalT[:, di, :],
                        start=(di == 0 and t in (0, 8, 16)), stop=False)

            # pair0 products interleaved with pair1-H1 adds/tanh/scores
            for di in range(9):
                if di >= 5:
                    tins_a[di] = emit_add(1, di, 0, TSPLIT)
                emit_tanh_scores(1, di, 0, TSPLIT, tins_a[di], psS1a,
                                 True, True)
                emit_product(0, di, 0, T, PM0[di], psbF, ppF, nc.sync)
            emit_exp(1, 0, TSPLIT, psS1a)
            # two pair1-H2 adds emitted early so tanh1-H2 never starves
            tins_b = {}
            for di in range(2):
                tins_b[di] = emit_add(1, di, TSPLIT, T)
            emit_softmax_transpose(1, 0, TSPLIT)
            for di in range(9):
                if di >= 2:
                    tins_b[di] = emit_add(1, di, TSPLIT, T)
                emit_tanh_scores(1, di, TSPLIT, T, tins_b[di], psS1b,
                                 True, True)
                emit_product(1, di, 0, TSPLIT, PM1A[di], psbH, ppH, nc.sync)
            emit_exp(1, TSPLIT, T, psS1b)
            emit_softmax_transpose(1, TSPLIT, T)
            for di in range(9):
                emit_product(1, di, TSPLIT, T, PM1B[di], psbH, ppH, nc.scalar)

            # ---- bias + leaky + out, pipelined per psum bank
            o_f = big.tile([128, T * 64], bf16d, tag="o_f", name="o_f")
            o_s2 = big.tile([128, T * 64], bf16d, tag="o_s2", name="o_s2")
            o_sb = big.tile([128, T * 64], bf16d, tag="o_sb", name="o_sb")
            for lo, hi in ((0, 8), (8, 16), (16, T)):
                for t in range(lo, hi):
                    nc.tensor.matmul(psO_all[:, t * 64:(t + 1) * 64],
                                     ones1, brow, start=False,
                                     stop=(t == hi - 1))
                r0, r1 = lo * 64, hi * 64
                nc.scalar.activation(out=o_f[:, r0:r1],
                                     in_=psO_all[:, r0:r1],
                                     func=AF.Identity)
                nc.vector.tensor_scalar(
                    out=o_s2[:, r0:r1], in0=o_f[:, r0:r1],
                    scalar1=0.2, scalar2=None, op0=OP.mult)
                nc.vector.tensor_max(o_sb[:, r0:r1], o_f[:, r0:r1],
                                     o_s2[:, r0:r1])
                nc.sync.dma_start(out=d_out[:, r0:r1], in_=o_sb[:, r0:r1])

    nc.compile()
    return nc


def _get_nc():
    if "nc" not in _CACHE:
        _CACHE["nc"] = build_nc()
    return _CACHE["nc"]


# ---------------------------------------------------------------- entry point
def _assemble(results):
    out = np.zeros((B, E, H, W), np.float32)
    s = np.arange(NW)
    pos = OB + s
    lr = pos // RP - 1
    c = pos % RP - 1
    sel = (c >= 0) & (c < W)
    for b in range(B):
        for q in range(4):
            res = np.asarray(results[b * 4 + q]["out"]).astype(np.float32)
            res = res.reshape(128, T, 64).transpose(1, 0, 2).reshape(T * 128, 64)
            out[b, :, q * ROWS_OUT + lr[sel], c[sel]] = res[s[sel]]
    return out


def kernel(**inputs):
    from concourse import bass_utils

    cores = host_prep(inputs)
    nc = _get_nc()
    res = bass_utils.run_bass_kernel_spmd(
        nc, [dict(c) for c in cores], core_ids=list(range(NCORES)))
    return _assemble(res.results)
